# revision 19
# baseline (speedup 1.0000x reference)
"""GroupNorm + 4-head self-attention + output projection, TRN2 Bass kernel.

Sharding: 8 cores = 4 batches x 2 query-halves.  Each core runs GroupNorm and
the full K/V projection for its batch (duplicated across the 2 cores of a
batch, ~5% extra FLOPs) and attention + output projection for its 2048-query
chunk.  The query chunk is rotated to the front of the token axis on the host
(GroupNorm stats / K / V are permutation-invariant along tokens), so all 8
cores run one identical SPMD program and the unshard is pure concatenation.

Device layout (per core).  The steady state is a 3-engine lockstep at
~1.4us per (j,i) iteration, all of PE/ACT/DVE ~92% busy:
  PE : 4 sim matmuls (row-tiled 4-up, concurrent; input-bus bound) +
       4 av + 4 dn matmuls (col-tiled 4-up) from 3 iterations back
  ACT: exact table exp of heads 0,1 ([128,2,512] per iteration)
  DVE: Schraudolph fast-exp of heads 2,3 -- one tensor_scalar computing
       round(sim * SCALE*log2e*128 + (127-c)*128) into a uint16 tile whose
       bits reinterpreted as bf16 equal exp(SCALE*sim) within +-3%; the
       softmax denominator cancels the systematic part (end-to-end ~6e-3,
       gate 2e-2).  The av/dn matmuls read it via .bitcast(bf16).
The dn matmuls use a dense all-ones [128,32] stationary so every partition
of the dn bank holds its head's denominator: full PE-tile utilization and
the epilogue needs no select/broadcast pass (Ln reads the psum directly).
PSUM: sim ring 3 slots x 2 banks + oacc 1 + dn 1 = 8 banks.  Exp/Ln/Square/
Copy/Identity are confined to the one ACT table set holding Exp+Ln, so
exactly one table load runs, at kernel start.

Per-j epilogue (5 pieces interleaved into the next j's iterations 2-6):
  lnd = Ln(dn psum) -> rcb = exp(-lnd) = 1/d on ACT (avoids the slow DVE
  iterative-divide reciprocal) -> ao = oacc*rcb on DVE -> per half:
  projection matmul, bias add on ACT (Identity + per-partition bias), DMA.

Prologue (~28us): x is uploaded bf16 in [128,512] pieces on BOTH hardware
DGE queues (t0 on sync, t1 on the ACT queue) with bn_stats chasing each
piece; both tiles' GroupNorm folds run as one batched chain of [*,2]-strided
ops (single Ln/Exp pair).  GroupNorm is FOLDED into the projections:
q = (Wq diag(alpha)) x + Wq beta, same for k; the v bias telescopes through
softmax (sum_m attn*vb = vb*denominator) into the output projection bias, so
normalized activations are never materialized.  K chunk 0 is emitted in a
128-column piece first so the first sim starts early; V is produced DIRECTLY
in the attention layout vS[m,o] by using the x chunk as the stationary
matmul operand (out = x_chunk^T @ Wv'), so no transposes of any kind exist
in the kernel.  Remaining K/V/Q chunks and the deferred output-bias fold are
emitted interleaved into j=0's i-loop.
"""

import numpy as np

HEAD = 4
DIM_HEAD = 32
DIM = 256
GROUPS = 32
EPS = 1e-5
SCALE = DIM_HEAD ** -0.5
N = 4096
NQ = 2048
NCORES = 8
P = 128
JW = 512           # query-chunk width per inner tile
NJ = NQ // JW      # 4
NI = N // P        # 32 key chunks

LOG2E = 1.4426950408889634
FE_A = float(SCALE * LOG2E * 128.0)      # fast-exp multiplier (scale folded)
FE_B = float((127.0 - 0.0430) * 128.0)   # fast-exp bias (Schraudolph c)

_cache = {}


def _get_nc():
    if "nc" in _cache:
        return _cache["nc"]
    from contextlib import ExitStack

    import concourse.bass as bass  # noqa: F401
    import concourse.tile as tile
    from concourse import bacc, mybir

    f32 = mybir.dt.float32
    b16 = mybir.dt.bfloat16
    u16 = mybir.dt.uint16
    AF = mybir.ActivationFunctionType
    ALU = mybir.AluOpType

    # Confine Exp/Ln to the one table set that holds both, so the table-load
    # pass never alternates sets (each switch costs ~1.3us of ACT time).
    # Membership-only edit: set order (= act_func_set_id) is preserved.
    import concourse.bacc as bacc_mod
    from concourse.hw_specs import get_activation_tables as _orig_tables

    def _tables_one_exp_ln_set(arch):
        combo = "natural_log_exp_and_others"
        out = {}
        for name, fns in _orig_tables(arch).items():
            if name != combo:
                fns = {f for f in fns
                       if f not in (AF.Exp, AF.Ln, AF.Square,
                                    AF.Copy, AF.Identity)}
            out[name] = fns
        return out

    bacc_mod.get_activation_tables = _tables_one_exp_ln_set

    nc = bacc.Bacc(None, target_bir_lowering=False)
    x_in = nc.declare_dram_parameter("x", [DIM, N], b16, isOutput=False)
    wqkvT = nc.declare_dram_parameter("wqkvT", [DIM, 3 * P], b16, isOutput=False)
    woutT = nc.declare_dram_parameter("woutT", [P, DIM], b16, isOutput=False)
    # small fp32 constants packed into one tensor / one DMA:
    # cols 0-1 gnw(t0,t1), 2-3 gnb, 4-5 bout, 6-21 blk8
    misc = nc.declare_dram_parameter("misc", [P, 22], f32, isOutput=False)
    blk8T = nc.declare_dram_parameter("blk8T", [16, P], f32, isOutput=False)
    y_out = nc.declare_dram_parameter("y", [DIM, NQ], f32, isOutput=True)

    with ExitStack() as ctx:
        tc = ctx.enter_context(tile.TileContext(nc))
        const = ctx.enter_context(tc.tile_pool(name="const", bufs=1))
        persist = ctx.enter_context(tc.tile_pool(name="persist", bufs=1))
        work = ctx.enter_context(tc.tile_pool(name="work", bufs=3))
        attnp = ctx.enter_context(tc.tile_pool(name="attnp", bufs=2))
        # PSUM budget (8 banks): sim ring 3 slots x 2 banks + oacc 1 + dn 1
        psA = ctx.enter_context(tc.tile_pool(name="psA", bufs=3, space="PSUM"))
        psB = ctx.enter_context(tc.tile_pool(name="psB", bufs=1, space="PSUM"))

        # ---- DMA order (one sync queue, issue-rate-bound): x t0 chunks
        # first with the GroupNorm stats chasing each chunk, then the small
        # consts, then x t1 (stats chasing), then the projection weights.
        xb = [persist.tile([P, N], b16, tag=f"xb{t}", name=f"xb{t}")
              for t in range(2)]
        stats = [work.tile([P, 8, 6], f32, tag=f"stats{t}", name=f"stats{t}")
                 for t in range(2)]

        # x upload on BOTH hardware DGE queues: t0 pieces on the sync
        # queue, t1 pieces on the ACT queue (idle during the prologue), so
        # the two tiles stream in parallel; bn_stats chases each piece.
        for ch in range(8):
            nc.sync.dma_start(
                out=xb[0][:, ch * 512:(ch + 1) * 512],
                in_=x_in[0:P, ch * 512:(ch + 1) * 512],
            )
            nc.scalar.dma_start(
                out=xb[1][:, ch * 512:(ch + 1) * 512],
                in_=x_in[P:2 * P, ch * 512:(ch + 1) * 512],
            )
        misc_sb = const.tile([P, 22], f32, tag="misc")
        nc.sync.dma_start(out=misc_sb, in_=misc[:, :])
        gnw_sb = misc_sb[:, 0:2]
        gnb_sb = misc_sb[:, 2:4]
        bout_sb = [misc_sb[:, 4 + t:5 + t] for t in range(2)]
        blk8_sb = misc_sb[:, 6:22]
        blk8T_sb = const.tile([16, P], f32, tag="blk8T")
        nc.sync.dma_start(out=blk8T_sb, in_=blk8T[:, :])
        wqkv_sb = []
        for t in range(2):
            w = const.tile([P, 3 * P], b16, tag=f"wqkv{t}", name=f"wqkv{t}")
            nc.sync.dma_start(out=w, in_=wqkvT[t * P:(t + 1) * P, :])
            wqkv_sb.append(w)
        wout_sb = const.tile([P, DIM], b16, tag="wout")
        nc.sync.dma_start(out=wout_sb, in_=woutT[:, :])

        ones32 = const.tile([P, 32], b16, tag="ones32")
        nc.vector.memset(ones32, 1.0)
        eps_sb = const.tile([16, 1], f32, tag="eps")
        nc.vector.memset(eps_sb, EPS)

        wqs = [persist.tile([P, 3 * P], b16, tag=f"wqs{t}", name=f"wqs{t}")
               for t in range(2)]
        be16 = [persist.tile([P, 1], b16, tag=f"be16{t}", name=f"be16{t}")
                for t in range(2)]
        qkvb_ps = psA.tile([P, 4], f32, tag="sim")

        # ---------------- GroupNorm ----------------
        # Stats chase the x DMA (emitted above); here: aggregate + the
        # per-group fold chain, t0 first (its stats arrive first).  The
        # per-channel scale (wqs) runs on ACT (Copy with per-partition
        # scale) so DVE can keep streaming t1's bn_stats.
        # Per-tile chains: t0's runs (mostly on ACT/PE) while DVE still
        # streams t1's bn_stats; t1's chain follows its aggregate.
        al2 = persist.tile([P, 2], f32, tag="al2")
        be2 = persist.tile([P, 2], f32, tag="be2")
        for t in range(2):
            # this tile's stats (t0's chain below is emitted before t1's
            # stats, so it executes while DVE still streams them; both
            # tiles' x pieces arrive in parallel on the two DGE queues)
            for ch in range(8):
                nc.vector.bn_stats(
                    out=stats[t][:, ch, :],
                    in_=xb[t][:, ch * 512:(ch + 1) * 512],
                )
            mv = work.tile([P, 2], f32, tag=f"mv{t}", name=f"mv{t}")
            nc.vector.bn_aggr(out=mv, in_=stats[t])
            msq = work.tile([P, 1], f32, tag=f"msq{t}", name=f"msq{t}")
            nc.vector.tensor_mul(msq, mv[:, 0:1], mv[:, 0:1])
            nc.vector.tensor_add(mv[:, 1:2], mv[:, 1:2], msq)
            gst_ps = psB.tile([16, 2], f32, tag="dn", name=f"gst_ps{t}")
            nc.tensor.matmul(gst_ps, lhsT=blk8_sb, rhs=mv,
                             start=True, stop=True)
            mmg = work.tile([16, 1], f32, tag=f"mmg{t}", name=f"mmg{t}")
            nc.scalar.activation(out=mmg, in_=gst_ps[:, 0:1], func=AF.Square)
            varg = work.tile([16, 1], f32, tag=f"varg{t}", name=f"varg{t}")
            nc.vector.tensor_sub(varg, gst_ps[:, 1:2], mmg)
            # rstd = exp(-0.5*ln(var+eps)): ln+exp share one ACT table set
            # with the attention exps (no extra ~2.7us table reload)
            sdg = work.tile([16, 1], f32, tag=f"sdg{t}", name=f"sdg{t}")
            nc.scalar.activation(
                out=sdg, in_=varg, func=AF.Ln, bias=eps_sb, scale=1.0
            )
            ms = work.tile([16, 2], f32, tag=f"ms{t}", name=f"ms{t}")
            nc.vector.tensor_copy(ms[:, 0:1], gst_ps[:, 0:1])
            nc.scalar.activation(
                out=ms[:, 1:2], in_=sdg, func=AF.Exp, scale=-0.5
            )
            cb_ps = psB.tile([P, 2], f32, tag="oacc", name=f"cb_ps{t}")
            nc.tensor.matmul(cb_ps, lhsT=blk8T_sb, rhs=ms,
                             start=True, stop=True)
            nc.vector.tensor_mul(al2[:, t:t + 1], cb_ps[:, 1:2],
                                 gnw_sb[:, t:t + 1])
            tmpb = work.tile([P, 1], f32, tag=f"tmpb{t}", name=f"tmpb{t}")
            nc.vector.tensor_mul(tmpb, cb_ps[:, 0:1], al2[:, t:t + 1])
            nc.vector.tensor_sub(be2[:, t:t + 1], gnb_sb[:, t:t + 1], tmpb)
            # fold the scale into this tile's projection weights right away
            if t == 0:
                nc.vector.tensor_scalar(out=wqs[0], in0=wqkv_sb[0],
                                        scalar1=al2[:, 0:1], scalar2=None,
                                        op0=ALU.mult)
            else:
                nc.scalar.activation(out=wqs[1], in_=wqkv_sb[1],
                                     func=AF.Copy, scale=al2[:, 1:2])
        albe = [(al2[:, t:t + 1], be2[:, t:t + 1]) for t in range(2)]
        # ---- fold GroupNorm into the projections: q = Wq'(x_bf) + qb,
        # Wq' = Wq diag(alpha), qb = Wq beta (same for k); the V bias
        # telescopes through attention (sum_m attn*vb = vb*denominator)
        # into the output projection bias: bout2 = bout + Wout vb.
        be16_2 = persist.tile([P, 2], b16, tag="be16_2")
        nc.scalar.activation(out=be16_2, in_=be2, func=AF.Copy)
        be16 = [be16_2[:, t:t + 1] for t in range(2)]
        for sel in range(3):
            for t in range(2):
                nc.tensor.matmul(
                    qkvb_ps[:, sel:sel + 1],
                    lhsT=wqkv_sb[t][:, sel * P:(sel + 1) * P],
                    rhs=be16[t], start=(t == 0), stop=(t == 1),
                )
        qb = persist.tile([P, 1], f32, tag="qb")
        nc.vector.tensor_copy(qb, qkvb_ps[:, 0:1])
        kb = persist.tile([P, 1], f32, tag="kb")
        nc.vector.tensor_copy(kb, qkvb_ps[:, 1:2])
        vb16 = persist.tile([P, 1], b16, tag="vb16")
        bout2 = [persist.tile([P, 1], f32, tag=f"bo2{t}", name=f"bo2{t}")
                 for t in range(2)]

        def emit_bout2():
            # deferred off the prologue critical path (first needed by the
            # j=0 epilogue, ~85us in)
            nc.vector.tensor_copy(vb16, qkvb_ps[:, 2:3])
            for t in range(2):
                bo_ps = psA.tile([P, 1], f32, tag="sim", name=f"bo_ps{t}")
                nc.tensor.matmul(bo_ps, lhsT=wout_sb[:, t * P:(t + 1) * P],
                                 rhs=vb16, start=True, stop=True)
                nc.vector.tensor_add(bout2[t], bo_ps, bout_sb[t])

        # ---------------- QKV projections ----------------
        qT = persist.tile([P, NQ], b16, tag="qT")
        kT = persist.tile([P, N], b16, tag="kT")
        vS = persist.tile([P, N], b16, tag="vS")   # vS[p, i*128+o] = v[i*128+p, o]

        def emit_q(jq):
            ps = psA.tile([P, 2, JW], f32, tag="sim")
            for t in range(2):
                nc.tensor.matmul(
                    ps[:, 0, :], lhsT=wqs[t][:, 0:P],
                    rhs=xb[t][:, jq * 512:(jq + 1) * 512],
                    start=(t == 0), stop=(t == 1),
                )
            nc.vector.tensor_scalar(out=qT[:, jq * 512:(jq + 1) * 512],
                                    in0=ps[:, 0, :], scalar1=qb,
                                    scalar2=None, op0=ALU.add)

        def emit_k(jk, splits=(512,)):
            base = jk * 512
            lo = 0
            for hi in splits:
                w = hi - lo
                ps = psA.tile([P, 2, JW], f32, tag="sim")
                for t in range(2):
                    nc.tensor.matmul(
                        ps[:, 0, 0:w], lhsT=wqs[t][:, P:2 * P],
                        rhs=xb[t][:, base + lo:base + hi],
                        start=(t == 0), stop=(t == 1),
                    )
                nc.scalar.activation(out=kT[:, base + lo:base + hi],
                                     in_=ps[:, 0, 0:w],
                                     func=AF.Identity, bias=kb, scale=1.0)
                lo = hi

        def emit_vS(ch):
            # one 512-token chunk of v, produced DIRECTLY in the attention
            # layout vS[m, o]: the x chunk is the stationary operand, so
            # out = x_chunk^T @ Wv' = v[m, o] -- no transposes needed.
            ps = psA.tile([P, 2, JW], f32, tag="sim", name="vsps")
            for blk in range(4):
                base = ch * 512 + blk * 128
                for t in range(2):
                    nc.tensor.matmul(
                        ps[:, 0, blk * 128:(blk + 1) * 128],
                        lhsT=xb[t][:, base:base + 128],
                        rhs=wqs[t][:, 2 * P:3 * P],
                        start=(t == 0), stop=(t == 1),
                    )
            if ch % 2 == 0:
                nc.scalar.activation(out=vS[:, ch * 512:(ch + 1) * 512],
                                     in_=ps[:, 0, :], func=AF.Copy)
            else:
                nc.vector.tensor_copy(vS[:, ch * 512:(ch + 1) * 512],
                                      ps[:, 0, :])

        # Produce only what attention j=0 needs up front; the rest (q 1-3,
        # k 1-7, v 4-31) is emitted interleaved into j=0's i-loop so the
        # first exp starts early.
        emit_k(0, splits=(128, 512))
        emit_q(0)

        # ---------------- attention ----------------
        # Per-j epilogue is emitted as 5 pieces interleaved into the first
        # iterations of the NEXT j (overlaps its serial chain with compute
        # and keeps the PE warm across the boundary).
        def make_epilogue(j, oacc, dn):
            def p0():
                # every partition of the dn bank already holds its head's
                # denominator (dense all-ones dn stationary), so ln reads
                # the psum bank directly -- no select/broadcast pass.
                lnd = work.tile([P, JW], f32, tag="lnd")
                nc.scalar.activation(out=lnd, in_=dn, func=AF.Ln)
                return lnd

            def p1(lnd):
                rcb = work.tile([P, JW], f32, tag="rcb")
                nc.scalar.activation(out=rcb, in_=lnd, func=AF.Exp, scale=-1.0)
                return rcb

            def p1b(rcb):
                ao = work.tile([P, JW], b16, tag="ao")
                nc.vector.tensor_mul(ao, oacc, rcb)
                return ao

            def p2(ao, t):
                yps = psA.tile([P, JW], f32, tag="sim")
                nc.tensor.matmul(
                    yps, lhsT=wout_sb[:, t * P:(t + 1) * P], rhs=ao,
                    start=True, stop=True,
                )
                ysb = work.tile([P, JW], f32, tag=f"ysb{t}", name=f"ysb{t}")
                # bias add on ACT (Identity with per-partition bias): keeps
                # the busier DVE free for the fexp stream
                nc.scalar.activation(out=ysb, in_=yps, func=AF.Identity,
                                     bias=bout2[t], scale=1.0)
                nc.sync.dma_start(
                    out=y_out[t * P:(t + 1) * P, j * JW:(j + 1) * JW], in_=ysb
                )

            state = {}

            def run_piece(k):
                if k == 0:
                    state["lnd"] = p0()
                elif k == 1:
                    state["rcb"] = p1(state["lnd"])
                elif k == 2:
                    state["ao"] = p1b(state["rcb"])
                elif k == 3:
                    p2(state["ao"], 0)
                elif k == 4:
                    p2(state["ao"], 1)

            def run_final():
                # last-j epilogue: nothing left to interleave with, so
                # pipeline it column-half by column-half (the h=1 chain
                # overlaps h=0's projection/bias/DMA stages)
                for h in range(2):
                    c0, c1 = h * 256, (h + 1) * 256
                    lnd = work.tile([P, 256], f32, tag=f"flnd{h}",
                                    name=f"flnd{h}")
                    nc.scalar.activation(out=lnd, in_=dn[:, c0:c1],
                                         func=AF.Ln)
                    rcb = work.tile([P, 256], f32, tag=f"frcb{h}",
                                    name=f"frcb{h}")
                    nc.scalar.activation(out=rcb, in_=lnd, func=AF.Exp,
                                         scale=-1.0)
                    ao = work.tile([P, 256], b16, tag=f"fao{h}",
                                   name=f"fao{h}")
                    nc.vector.tensor_mul(ao, oacc[:, c0:c1], rcb)
                    for t in range(2):
                        yps = psA.tile([P, JW], f32, tag="sim")
                        nc.tensor.matmul(
                            yps[:, 0:256],
                            lhsT=wout_sb[:, t * P:(t + 1) * P], rhs=ao,
                            start=True, stop=True,
                        )
                        ysb = work.tile([P, 256], f32, tag=f"fysb{t}{h}",
                                        name=f"fysb{t}{h}")
                        if t == 0:
                            nc.scalar.activation(out=ysb, in_=yps[:, 0:256],
                                                 func=AF.Identity,
                                                 bias=bout2[t], scale=1.0)
                        else:
                            nc.vector.tensor_scalar_add(ysb, yps[:, 0:256],
                                                        bout2[t])
                        nc.sync.dma_start(
                            out=y_out[t * P:(t + 1) * P,
                                      j * JW + c0:j * JW + c1],
                            in_=ysb,
                        )

            run_piece.final = run_final
            return run_piece

        NPIECE = 5
        EPI_AT = (2, 3, 4, 5, 6)
        AVDELAY = 3
        epilogue = None
        pending = []        # av/dn emission pipeline, carried ACROSS j
        for j in range(NJ):
            oacc = psB.tile([P, JW], f32, tag="oacc")
            dn = psB.tile([P, JW], f32, tag="dn")

            def emit_avdn(i, at0, at1, oacc=oacc, dn=dn):
                rhss = [at0[:, 0, :], at0[:, 1, :],
                        at1[:, 0, :].bitcast(b16), at1[:, 1, :].bitcast(b16)]
                for h in range(HEAD):
                    nc.tensor.matmul(
                        oacc[32 * h:32 * h + 32, :],
                        lhsT=vS[:, i * P + 32 * h:i * P + 32 * h + 32],
                        rhs=rhss[h],
                        start=(i == 0), stop=(i == NI - 1),
                        tile_position=(0, 32 * h),
                        skip_group_check=True,
                    )
                for h in range(HEAD):
                    # dense all-ones stationary: all 32 partitions of each
                    # head's dn block receive the denominator (broadcast
                    # done by the PE for free; full tile utilization).
                    nc.tensor.matmul(
                        dn[32 * h:32 * h + 32, :],
                        lhsT=ones32,
                        rhs=rhss[h],
                        start=(i == 0), stop=(i == NI - 1),
                        tile_position=(0, 32 * h),
                        skip_group_check=True,
                    )

            for i in range(NI):
                if j == 0:
                    # k and vS chunk emissions on alternating iterations so
                    # no single iteration carries a double insertion into
                    # the exp streams
                    if i == 0:
                        emit_vS(0)
                    elif i % 4 == 1 and i <= 25:
                        emit_k((i + 3) // 4)
                    elif i % 4 == 3 and 3 <= i <= 27:
                        emit_vS((i + 1) // 4)
                    elif i == 28:
                        emit_bout2()
                    if i in (2, 4, 6):
                        emit_q(i // 2)
                sims = []
                for pr in range(2):
                    sim = psA.tile([P, 2, JW], f32, tag="sim")
                    for hh in range(2):
                        h = pr * 2 + hh
                        nc.tensor.matmul(
                            sim[:, hh, :],
                            lhsT=kT[32 * h:32 * h + 32, i * P:(i + 1) * P],
                            rhs=qT[32 * h:32 * h + 32, j * JW:(j + 1) * JW],
                            start=True, stop=True,
                            tile_position=(32 * h, 0),
                        )
                    sims.append(sim)
                # heads 0,1: exact exp on ACT; heads 2,3: fast-exp on DVE
                at0 = attnp.tile([P, 2, JW], b16, tag="at0", bufs=7)
                nc.scalar.activation(out=at0, in_=sims[0], func=AF.Exp,
                                     scale=SCALE)
                at1 = attnp.tile([P, 2, JW], u16, tag="at1", bufs=7)
                nc.vector.tensor_scalar(
                    out=at1, in0=sims[1], scalar1=FE_A, scalar2=FE_B,
                    op0=ALU.mult, op1=ALU.add,
                )
                pending.append((emit_avdn, i, at0, at1))
                if len(pending) > AVDELAY:
                    fn, ii, a0, a1 = pending.pop(0)
                    fn(ii, a0, a1)
                if epilogue is not None and i in EPI_AT:
                    epilogue(EPI_AT.index(i))
                    if i == EPI_AT[-1]:
                        epilogue = None
            epilogue = make_epilogue(j, oacc, dn)
        for fn, ii, a0, a1 in pending:
            fn(ii, a0, a1)
        epilogue.final()

    nc.finalize()
    _cache["nc"] = nc
    return nc


def _prep_in_maps(x, gn_weight, gn_bias, w_qkv, w_out, b_out):
    import ml_dtypes

    f = np.float32
    bf = ml_dtypes.bfloat16
    x = np.asarray(x, dtype=f).astype(bf)
    wqkvT = np.ascontiguousarray(np.asarray(w_qkv, dtype=f).T.astype(bf))
    woutT = np.ascontiguousarray(np.asarray(w_out, dtype=f).T.astype(bf))
    gnw = np.asarray(gn_weight, dtype=f).reshape(2, P)
    gnb = np.asarray(gn_bias, dtype=f).reshape(2, P)
    bo = np.asarray(b_out, dtype=f).reshape(2, P)
    ar = np.arange(P)
    # misc pack: cols 0-1 gnw(t0,t1), 2-3 gnb, 4-5 unused, 6-21 blk8
    misc = np.zeros((P, 22), f)
    misc[:, 0] = gnw[0]
    misc[:, 1] = gnw[1]
    misc[:, 2] = gnb[0]
    misc[:, 3] = gnb[1]
    misc[:, 4] = bo[0]
    misc[:, 5] = bo[1]
    misc[ar, 6 + ar // 8] = 0.125
    blk8T = np.zeros((16, P), f)
    blk8T[ar // 8, ar] = 1.0
    shared = dict(wqkvT=wqkvT, woutT=woutT, misc=misc, blk8T=blk8T)
    in_maps = []
    for core in range(NCORES):
        b, half = divmod(core, 2)
        xb = x[b].reshape(DIM, N)
        if half == 0:
            xp = np.ascontiguousarray(xb)
        else:
            xp = np.ascontiguousarray(
                np.concatenate([xb[:, NQ:], xb[:, :NQ]], axis=1)
            )
        in_maps.append(dict(x=xp, **shared))
    return in_maps


def _get_executor():
    """Build the sharded jitted executor once (compiles the NEFF once).

    Returns (exec_fn, meta): exec_fn takes a list of 8 per-core input dicts
    and returns the list of 8 per-core output dicts.  Mirrors
    concourse.bass2jax.run_bass_via_pjrt's multi-core path but caches the
    jax.jit so repeated calls don't recompile.
    """
    if "exec" in _cache:
        return _cache["exec"]
    import jax
    import concourse.mybir as mybir
    from jax.sharding import Mesh, PartitionSpec
    from jax.experimental.shard_map import shard_map
    from concourse import bass2jax

    bass2jax.install_neuronx_cc_hook()
    nc = _get_nc()

    partition_name = (
        nc.partition_id_tensor.name if nc.partition_id_tensor else None
    )
    in_names, out_names, out_avals, zero_outs = [], [], [], []
    for alloc in nc.m.functions[0].allocations:
        if not isinstance(alloc, mybir.MemoryLocationSet):
            continue
        name = alloc.memorylocations[0].name
        if alloc.kind == "ExternalInput":
            if name != partition_name:
                in_names.append(name)
        elif alloc.kind == "ExternalOutput":
            shape = tuple(alloc.tensor_shape)
            dtype = mybir.dt.np(alloc.dtype)
            out_names.append(name)
            out_avals.append(jax.core.ShapedArray(shape, dtype))
            zero_outs.append(np.zeros(shape, dtype))
    n_params = len(in_names)
    n_outs = len(out_names)
    all_names = in_names + out_names
    if partition_name is not None:
        all_names = all_names + [partition_name]

    def _body(*args):
        operands = list(args)
        if partition_name is not None:
            operands.append(bass2jax.partition_id_tensor())
        outs = bass2jax._bass_exec_p.bind(
            *operands,
            out_avals=tuple(out_avals),
            in_names=tuple(all_names),
            out_names=tuple(out_names),
            lowering_input_output_aliases=(),
            sim_require_finite=True,
            sim_require_nnan=True,
            nc=nc,
        )
        return tuple(outs)

    devices = jax.devices()[:NCORES]
    mesh = Mesh(np.asarray(devices), ("core",))
    sharded = jax.jit(
        shard_map(
            _body, mesh=mesh,
            in_specs=(PartitionSpec("core"),) * (n_params + n_outs),
            out_specs=(PartitionSpec("core"),) * n_outs,
            check_rep=False,
        ),
        keep_unused=True,
    )
    from jax.sharding import NamedSharding
    sharding = NamedSharding(mesh, PartitionSpec("core"))
    dev_zeros = [
        jax.device_put(
            np.zeros((NCORES * z.shape[0], *z.shape[1:]), z.dtype), sharding
        )
        for z in zero_outs
    ]

    def put_inputs(in_maps):
        return [
            jax.device_put(
                np.concatenate([np.asarray(m[name]) for m in in_maps], axis=0),
                sharding,
            )
            for name in in_names
        ]

    def run_device(device_inputs):
        return sharded(*device_inputs, *dev_zeros)

    def exec_fn(in_maps, device_inputs=None):
        if device_inputs is None:
            device_inputs = put_inputs(in_maps)
        out_arrs = [np.asarray(a) for a in run_device(device_inputs)]
        return [
            {
                name: out_arrs[i].reshape(NCORES, *out_avals[i].shape)[c]
                for i, name in enumerate(out_names)
            }
            for c in range(NCORES)
        ]

    meta = dict(in_names=in_names, out_names=out_names, mesh=mesh,
                sharded=sharded, zero_outs=zero_outs,
                put_inputs=put_inputs, run_device=run_device)
    _cache["exec"] = (exec_fn, meta)
    return _cache["exec"]


def _assemble(results):
    y = np.empty((4, DIM, N), np.float32)
    for core in range(NCORES):
        b, half = divmod(core, 2)
        y[b][:, half * NQ:(half + 1) * NQ] = results[core]["y"]
    return y.reshape(4, DIM, 64, 64)


def _run(inputs, **kw):
    exec_fn, _ = _get_executor()
    in_maps = _prep_in_maps(**inputs)
    results = exec_fn(in_maps)
    return _assemble(results), results


def kernel(x, gn_weight, gn_bias, w_qkv, w_out, b_out):
    out, _ = _run(dict(x=x, gn_weight=gn_weight, gn_bias=gn_bias,
                       w_qkv=w_qkv, w_out=w_out, b_out=b_out))
    return out


# revision 20
# speedup vs baseline: 1.0423x; 1.0423x over previous
"""GroupNorm + 4-head self-attention + output projection, TRN2 Bass kernel.

Sharding: 8 cores = 4 batches x 2 query-halves.  Each core runs GroupNorm and
the full K/V projection for its batch (duplicated across the 2 cores of a
batch, ~5% extra FLOPs) and attention + output projection for its 2048-query
chunk.  The query chunk is rotated to the front of the token axis on the host
(GroupNorm stats / K / V are permutation-invariant along tokens), so all 8
cores run one identical SPMD program and the unshard is pure concatenation.

Device layout (per core).  The steady state is a 3-engine lockstep at
~1.4us per (j,i) iteration, all of PE/ACT/DVE ~92% busy:
  PE : 4 sim matmuls (row-tiled 4-up, concurrent; input-bus bound) +
       4 av + 4 dn matmuls (col-tiled 4-up) from 3 iterations back
  ACT: exact table exp of heads 0,1 ([128,2,512] per iteration)
  DVE: Schraudolph fast-exp of heads 2,3 -- one tensor_scalar computing
       round(sim * SCALE*log2e*128 + (127-c)*128) into a uint16 tile whose
       bits reinterpreted as bf16 equal exp(SCALE*sim) within +-3%; the
       softmax denominator cancels the systematic part (end-to-end ~6e-3,
       gate 2e-2).  The av/dn matmuls read it via .bitcast(bf16).
The dn matmuls use a dense all-ones [128,32] stationary so every partition
of the dn bank holds its head's denominator: full PE-tile utilization and
the epilogue needs no select/broadcast pass (Ln reads the psum directly).
PSUM: sim ring 3 slots x 2 banks + oacc 1 + dn 1 = 8 banks.  Exp/Ln/Square/
Copy/Identity are confined to the one ACT table set holding Exp+Ln, so
exactly one table load runs, at kernel start.

Per-j epilogue (5 pieces interleaved into the next j's iterations 2-6):
  lnd = Ln(dn psum) -> rcb = exp(-lnd) = 1/d on ACT (avoids the slow DVE
  iterative-divide reciprocal) -> ao = oacc*rcb on DVE -> per half:
  projection matmul, bias add on ACT (Identity + per-partition bias), DMA.

Prologue (~28us): x is uploaded bf16 in [128,512] pieces on BOTH hardware
DGE queues (t0 on sync, t1 on the ACT queue) with bn_stats chasing each
piece; both tiles' GroupNorm folds run as one batched chain of [*,2]-strided
ops (single Ln/Exp pair).  GroupNorm is FOLDED into the projections:
q = (Wq diag(alpha)) x + Wq beta, same for k; the v bias telescopes through
softmax (sum_m attn*vb = vb*denominator) into the output projection bias, so
normalized activations are never materialized.  K chunk 0 is emitted in a
128-column piece first so the first sim starts early; V is produced DIRECTLY
in the attention layout vS[m,o] by using the x chunk as the stationary
matmul operand (out = x_chunk^T @ Wv'), so no transposes of any kind exist
in the kernel.  Remaining K/V/Q chunks and the deferred output-bias fold are
emitted interleaved into j=0's i-loop.
"""

import numpy as np

HEAD = 4
DIM_HEAD = 32
DIM = 256
GROUPS = 32
EPS = 1e-5
SCALE = DIM_HEAD ** -0.5
N = 4096
NQ = 2048
NCORES = 8
P = 128
JW = 512           # query-chunk width per inner tile
NJ = NQ // JW      # 4
NI = N // P        # 32 key chunks

LOG2E = 1.4426950408889634
FE_A = float(SCALE * LOG2E * 128.0)      # fast-exp multiplier (scale folded)
FE_B = float((127.0 - 0.0430) * 128.0)   # fast-exp bias (Schraudolph c)

_cache = {}


def _get_nc():
    if "nc" in _cache:
        return _cache["nc"]
    from contextlib import ExitStack

    import concourse.bass as bass  # noqa: F401
    import concourse.tile as tile
    from concourse import bacc, mybir

    f32 = mybir.dt.float32
    b16 = mybir.dt.bfloat16
    u16 = mybir.dt.uint16
    AF = mybir.ActivationFunctionType
    ALU = mybir.AluOpType

    # Confine Exp/Ln to the one table set that holds both, so the table-load
    # pass never alternates sets (each switch costs ~1.3us of ACT time).
    # Membership-only edit: set order (= act_func_set_id) is preserved.
    import concourse.bacc as bacc_mod
    from concourse.hw_specs import get_activation_tables as _orig_tables

    def _tables_one_exp_ln_set(arch):
        combo = "natural_log_exp_and_others"
        out = {}
        for name, fns in _orig_tables(arch).items():
            if name != combo:
                fns = {f for f in fns
                       if f not in (AF.Exp, AF.Ln, AF.Square,
                                    AF.Copy, AF.Identity)}
            out[name] = fns
        return out

    bacc_mod.get_activation_tables = _tables_one_exp_ln_set

    nc = bacc.Bacc(None, target_bir_lowering=False)
    x_in = nc.declare_dram_parameter("x", [DIM, N], b16, isOutput=False)
    wqkvT = nc.declare_dram_parameter("wqkvT", [DIM, 3 * P], b16, isOutput=False)
    woutT = nc.declare_dram_parameter("woutT", [P, DIM], b16, isOutput=False)
    # small fp32 constants packed into one tensor / one DMA:
    # cols 0-1 gnw(t0,t1), 2-3 gnb, 4-5 bout, 6-21 blk8
    misc = nc.declare_dram_parameter("misc", [P, 22], f32, isOutput=False)
    blk8T = nc.declare_dram_parameter("blk8T", [16, P], f32, isOutput=False)
    y_out = nc.declare_dram_parameter("y", [DIM, NQ], f32, isOutput=True)

    with ExitStack() as ctx:
        tc = ctx.enter_context(tile.TileContext(nc))
        const = ctx.enter_context(tc.tile_pool(name="const", bufs=1))
        persist = ctx.enter_context(tc.tile_pool(name="persist", bufs=1))
        work = ctx.enter_context(tc.tile_pool(name="work", bufs=3))
        attnp = ctx.enter_context(tc.tile_pool(name="attnp", bufs=2))
        # PSUM budget (8 banks): sim ring 3 slots x 2 banks + oacc 1 + dn 1
        psA = ctx.enter_context(tc.tile_pool(name="psA", bufs=3, space="PSUM"))
        psB = ctx.enter_context(tc.tile_pool(name="psB", bufs=1, space="PSUM"))

        # ---- DMA order (one sync queue, issue-rate-bound): x t0 chunks
        # first with the GroupNorm stats chasing each chunk, then the small
        # consts, then x t1 (stats chasing), then the projection weights.
        xb = [persist.tile([P, N], b16, tag=f"xb{t}", name=f"xb{t}")
              for t in range(2)]
        stats = [work.tile([P, 8, 6], f32, tag=f"stats{t}", name=f"stats{t}")
                 for t in range(2)]

        # x upload on BOTH hardware DGE queues: t0 pieces on the sync
        # queue, t1 pieces on the ACT queue (idle during the prologue), so
        # the two tiles stream in parallel; bn_stats chases each piece.
        for ch in range(8):
            nc.sync.dma_start(
                out=xb[0][:, ch * 512:(ch + 1) * 512],
                in_=x_in[0:P, ch * 512:(ch + 1) * 512],
            )
            nc.scalar.dma_start(
                out=xb[1][:, ch * 512:(ch + 1) * 512],
                in_=x_in[P:2 * P, ch * 512:(ch + 1) * 512],
            )
        misc_sb = const.tile([P, 22], f32, tag="misc")
        nc.sync.dma_start(out=misc_sb, in_=misc[:, :])
        gnw_sb = misc_sb[:, 0:2]
        gnb_sb = misc_sb[:, 2:4]
        bout_sb = [misc_sb[:, 4 + t:5 + t] for t in range(2)]
        blk8_sb = misc_sb[:, 6:22]
        blk8T_sb = const.tile([16, P], f32, tag="blk8T")
        nc.sync.dma_start(out=blk8T_sb, in_=blk8T[:, :])
        wqkv_sb = []
        for t in range(2):
            w = const.tile([P, 3 * P], b16, tag=f"wqkv{t}", name=f"wqkv{t}")
            nc.sync.dma_start(out=w, in_=wqkvT[t * P:(t + 1) * P, :])
            wqkv_sb.append(w)
        wout_sb = const.tile([P, DIM], b16, tag="wout")
        nc.sync.dma_start(out=wout_sb, in_=woutT[:, :])
        # stats in piece-arrival order: t0/t1 pieces land in parallel, so
        # interleave t0/t1 on DVE to chase both queues.
        for ch in range(8):
            for t in range(2):
                nc.vector.bn_stats(
                    out=stats[t][:, ch, :],
                    in_=xb[t][:, ch * 512:(ch + 1) * 512],
                )
        ones32 = const.tile([P, 32], b16, tag="ones32")
        nc.vector.memset(ones32, 1.0)
        eps_sb = const.tile([16, 1], f32, tag="eps")
        nc.vector.memset(eps_sb, EPS)

        wqs = [persist.tile([P, 3 * P], b16, tag=f"wqs{t}", name=f"wqs{t}")
               for t in range(2)]
        be16 = [persist.tile([P, 1], b16, tag=f"be16{t}", name=f"be16{t}")
                for t in range(2)]
        qkvb_ps = psA.tile([P, 4], f32, tag="sim")

        # ---------------- GroupNorm ----------------
        # Stats chase the x DMA (emitted above); here: aggregate + the
        # per-group fold chain, t0 first (its stats arrive first).  The
        # per-channel scale (wqs) runs on ACT (Copy with per-partition
        # scale) so DVE can keep streaming t1's bn_stats.
        # Both tiles' chains batched into one set of [*, 2]-strided ops:
        # mv4 cols = (mean0, Ex2_0, mean1, Ex2_1).
        mv4 = work.tile([P, 4], f32, tag="mv4")
        for t in range(2):
            nc.vector.bn_aggr(out=mv4[:, 2 * t:2 * t + 2], in_=stats[t])
        msq = work.tile([P, 2], f32, tag="msq")
        nc.vector.tensor_mul(msq, mv4[:, 0:4:2], mv4[:, 0:4:2])
        nc.vector.tensor_add(mv4[:, 1:4:2], mv4[:, 1:4:2], msq)
        # per-group (mean, E[x^2]) for both tiles in one matmul
        gst_ps = psB.tile([16, 4], f32, tag="dn", name="gst_ps")
        nc.tensor.matmul(gst_ps, lhsT=blk8_sb, rhs=mv4, start=True, stop=True)
        mmg = work.tile([16, 2], f32, tag="mmg")
        nc.scalar.activation(out=mmg, in_=gst_ps[:, 0:4:2], func=AF.Square)
        varg = work.tile([16, 2], f32, tag="varg")
        nc.vector.tensor_sub(varg, gst_ps[:, 1:4:2], mmg)
        # rstd = exp(-0.5*ln(var+eps)): ln+exp share one ACT table set
        # with the attention exps (no extra ~2.7us table reload)
        sdg = work.tile([16, 2], f32, tag="sdg")
        nc.scalar.activation(
            out=sdg, in_=varg, func=AF.Ln, bias=eps_sb, scale=1.0
        )
        ms4 = work.tile([16, 4], f32, tag="ms4")
        nc.vector.tensor_copy(ms4[:, 0:4:2], gst_ps[:, 0:4:2])
        nc.scalar.activation(
            out=ms4[:, 1:4:2], in_=sdg, func=AF.Exp, scale=-0.5
        )
        # broadcast group (mean, rstd) to the 8 channels of each group
        cb_ps = psB.tile([P, 4], f32, tag="oacc", name="cb_ps")
        nc.tensor.matmul(cb_ps, lhsT=blk8T_sb, rhs=ms4,
                         start=True, stop=True)
        al2 = persist.tile([P, 2], f32, tag="al2")
        nc.vector.tensor_mul(al2, cb_ps[:, 1:4:2], gnw_sb)
        tmpb = work.tile([P, 2], f32, tag="tmpb")
        nc.vector.tensor_mul(tmpb, cb_ps[:, 0:4:2], al2)
        be2 = persist.tile([P, 2], f32, tag="be2")
        nc.vector.tensor_sub(be2, gnb_sb, tmpb)
        albe = [(al2[:, t:t + 1], be2[:, t:t + 1]) for t in range(2)]
        # ---- fold GroupNorm into the projections: q = Wq'(x_bf) + qb,
        # Wq' = Wq diag(alpha), qb = Wq beta (same for k); the V bias
        # telescopes through attention (sum_m attn*vb = vb*denominator)
        # into the output projection bias: bout2 = bout + Wout vb.
        be16_2 = persist.tile([P, 2], b16, tag="be16_2")
        nc.scalar.activation(out=be16_2, in_=be2, func=AF.Copy)
        be16 = [be16_2[:, t:t + 1] for t in range(2)]
        # t0 scale on DVE (bf16 SBUF operands: 4x mode), t1 on ACT - parallel
        nc.vector.tensor_scalar(out=wqs[0], in0=wqkv_sb[0],
                                scalar1=albe[0][0], scalar2=None,
                                op0=ALU.mult)
        nc.scalar.activation(out=wqs[1], in_=wqkv_sb[1], func=AF.Copy,
                             scale=albe[1][0])
        for sel in range(3):
            for t in range(2):
                nc.tensor.matmul(
                    qkvb_ps[:, sel:sel + 1],
                    lhsT=wqkv_sb[t][:, sel * P:(sel + 1) * P],
                    rhs=be16[t], start=(t == 0), stop=(t == 1),
                )
        qb = persist.tile([P, 1], f32, tag="qb")
        nc.vector.tensor_copy(qb, qkvb_ps[:, 0:1])
        kb = persist.tile([P, 1], f32, tag="kb")
        nc.vector.tensor_copy(kb, qkvb_ps[:, 1:2])
        vb16 = persist.tile([P, 1], b16, tag="vb16")
        bout2 = [persist.tile([P, 1], f32, tag=f"bo2{t}", name=f"bo2{t}")
                 for t in range(2)]

        def emit_bout2():
            # deferred off the prologue critical path (first needed by the
            # j=0 epilogue, ~85us in)
            nc.vector.tensor_copy(vb16, qkvb_ps[:, 2:3])
            for t in range(2):
                bo_ps = psA.tile([P, 1], f32, tag="sim", name=f"bo_ps{t}")
                nc.tensor.matmul(bo_ps, lhsT=wout_sb[:, t * P:(t + 1) * P],
                                 rhs=vb16, start=True, stop=True)
                nc.vector.tensor_add(bout2[t], bo_ps, bout_sb[t])

        # ---------------- QKV projections ----------------
        qT = persist.tile([P, NQ], b16, tag="qT")
        kT = persist.tile([P, N], b16, tag="kT")
        vS = persist.tile([P, N], b16, tag="vS")   # vS[p, i*128+o] = v[i*128+p, o]

        def emit_q(jq):
            ps = psA.tile([P, 2, JW], f32, tag="sim")
            for t in range(2):
                nc.tensor.matmul(
                    ps[:, 0, :], lhsT=wqs[t][:, 0:P],
                    rhs=xb[t][:, jq * 512:(jq + 1) * 512],
                    start=(t == 0), stop=(t == 1),
                )
            nc.vector.tensor_scalar(out=qT[:, jq * 512:(jq + 1) * 512],
                                    in0=ps[:, 0, :], scalar1=qb,
                                    scalar2=None, op0=ALU.add)

        def emit_k(jk, splits=(512,)):
            base = jk * 512
            lo = 0
            for hi in splits:
                w = hi - lo
                ps = psA.tile([P, 2, JW], f32, tag="sim")
                for t in range(2):
                    nc.tensor.matmul(
                        ps[:, 0, 0:w], lhsT=wqs[t][:, P:2 * P],
                        rhs=xb[t][:, base + lo:base + hi],
                        start=(t == 0), stop=(t == 1),
                    )
                nc.scalar.activation(out=kT[:, base + lo:base + hi],
                                     in_=ps[:, 0, 0:w],
                                     func=AF.Identity, bias=kb, scale=1.0)
                lo = hi

        def emit_vS(ch):
            # one 512-token chunk of v, produced DIRECTLY in the attention
            # layout vS[m, o]: the x chunk is the stationary operand, so
            # out = x_chunk^T @ Wv' = v[m, o] -- no transposes needed.
            ps = psA.tile([P, 2, JW], f32, tag="sim", name="vsps")
            for blk in range(4):
                base = ch * 512 + blk * 128
                for t in range(2):
                    nc.tensor.matmul(
                        ps[:, 0, blk * 128:(blk + 1) * 128],
                        lhsT=xb[t][:, base:base + 128],
                        rhs=wqs[t][:, 2 * P:3 * P],
                        start=(t == 0), stop=(t == 1),
                    )
            if ch % 2 == 0:
                nc.scalar.activation(out=vS[:, ch * 512:(ch + 1) * 512],
                                     in_=ps[:, 0, :], func=AF.Copy)
            else:
                nc.vector.tensor_copy(vS[:, ch * 512:(ch + 1) * 512],
                                      ps[:, 0, :])

        # Produce only what attention j=0 needs up front; the rest (q 1-3,
        # k 1-7, v 4-31) is emitted interleaved into j=0's i-loop so the
        # first exp starts early.
        emit_k(0, splits=(128, 512))
        emit_q(0)

        # ---------------- attention ----------------
        # Per-j epilogue is emitted as 5 pieces interleaved into the first
        # iterations of the NEXT j (overlaps its serial chain with compute
        # and keeps the PE warm across the boundary).
        def make_epilogue(j, oacc, dn):
            def p0():
                # every partition of the dn bank already holds its head's
                # denominator (dense all-ones dn stationary), so ln reads
                # the psum bank directly -- no select/broadcast pass.
                lnd = work.tile([P, JW], f32, tag="lnd")
                nc.scalar.activation(out=lnd, in_=dn, func=AF.Ln)
                return lnd

            def p1(lnd):
                rcb = work.tile([P, JW], f32, tag="rcb")
                nc.scalar.activation(out=rcb, in_=lnd, func=AF.Exp, scale=-1.0)
                return rcb

            def p1b(rcb):
                ao = work.tile([P, JW], b16, tag="ao")
                nc.vector.tensor_mul(ao, oacc, rcb)
                return ao

            def p2(ao, t):
                yps = psA.tile([P, JW], f32, tag="sim")
                nc.tensor.matmul(
                    yps, lhsT=wout_sb[:, t * P:(t + 1) * P], rhs=ao,
                    start=True, stop=True,
                )
                ysb = work.tile([P, JW], f32, tag=f"ysb{t}", name=f"ysb{t}")
                # bias add on ACT (Identity with per-partition bias): keeps
                # the busier DVE free for the fexp stream
                nc.scalar.activation(out=ysb, in_=yps, func=AF.Identity,
                                     bias=bout2[t], scale=1.0)
                nc.sync.dma_start(
                    out=y_out[t * P:(t + 1) * P, j * JW:(j + 1) * JW], in_=ysb
                )

            state = {}

            def run_piece(k):
                if k == 0:
                    state["lnd"] = p0()
                elif k == 1:
                    state["rcb"] = p1(state["lnd"])
                elif k == 2:
                    state["ao"] = p1b(state["rcb"])
                elif k == 3:
                    p2(state["ao"], 0)
                elif k == 4:
                    p2(state["ao"], 1)

            def run_final():
                # last-j epilogue: nothing left to interleave with, so
                # pipeline it column-half by column-half (the h=1 chain
                # overlaps h=0's projection/bias/DMA stages)
                for h in range(2):
                    c0, c1 = h * 256, (h + 1) * 256
                    lnd = work.tile([P, 256], f32, tag=f"flnd{h}",
                                    name=f"flnd{h}")
                    nc.scalar.activation(out=lnd, in_=dn[:, c0:c1],
                                         func=AF.Ln)
                    rcb = work.tile([P, 256], f32, tag=f"frcb{h}",
                                    name=f"frcb{h}")
                    nc.scalar.activation(out=rcb, in_=lnd, func=AF.Exp,
                                         scale=-1.0)
                    ao = work.tile([P, 256], b16, tag=f"fao{h}",
                                   name=f"fao{h}")
                    nc.vector.tensor_mul(ao, oacc[:, c0:c1], rcb)
                    for t in range(2):
                        yps = psA.tile([P, JW], f32, tag="sim")
                        nc.tensor.matmul(
                            yps[:, 0:256],
                            lhsT=wout_sb[:, t * P:(t + 1) * P], rhs=ao,
                            start=True, stop=True,
                        )
                        ysb = work.tile([P, 256], f32, tag=f"fysb{t}{h}",
                                        name=f"fysb{t}{h}")
                        if t == 0:
                            nc.scalar.activation(out=ysb, in_=yps[:, 0:256],
                                                 func=AF.Identity,
                                                 bias=bout2[t], scale=1.0)
                        else:
                            nc.vector.tensor_scalar_add(ysb, yps[:, 0:256],
                                                        bout2[t])
                        nc.sync.dma_start(
                            out=y_out[t * P:(t + 1) * P,
                                      j * JW + c0:j * JW + c1],
                            in_=ysb,
                        )

            run_piece.final = run_final
            return run_piece

        NPIECE = 5
        EPI_AT = (2, 3, 4, 5, 6)
        AVDELAY = 3
        epilogue = None
        pending = []        # av/dn emission pipeline, carried ACROSS j
        for j in range(NJ):
            oacc = psB.tile([P, JW], f32, tag="oacc")
            dn = psB.tile([P, JW], f32, tag="dn")

            def emit_avdn(i, at0, at1, oacc=oacc, dn=dn):
                rhss = [at0[:, 0, :], at0[:, 1, :],
                        at1[:, 0, :].bitcast(b16), at1[:, 1, :].bitcast(b16)]
                for h in range(HEAD):
                    nc.tensor.matmul(
                        oacc[32 * h:32 * h + 32, :],
                        lhsT=vS[:, i * P + 32 * h:i * P + 32 * h + 32],
                        rhs=rhss[h],
                        start=(i == 0), stop=(i == NI - 1),
                        tile_position=(0, 32 * h),
                        skip_group_check=True,
                    )
                for h in range(HEAD):
                    # dense all-ones stationary: all 32 partitions of each
                    # head's dn block receive the denominator (broadcast
                    # done by the PE for free; full tile utilization).
                    nc.tensor.matmul(
                        dn[32 * h:32 * h + 32, :],
                        lhsT=ones32,
                        rhs=rhss[h],
                        start=(i == 0), stop=(i == NI - 1),
                        tile_position=(0, 32 * h),
                        skip_group_check=True,
                    )

            for i in range(NI):
                if j == 0:
                    # k and vS chunk emissions on alternating iterations so
                    # no single iteration carries a double insertion into
                    # the exp streams
                    if i == 0:
                        emit_vS(0)
                    elif i % 4 == 1 and i <= 25:
                        emit_k((i + 3) // 4)
                    elif i % 4 == 3 and 3 <= i <= 27:
                        emit_vS((i + 1) // 4)
                    elif i == 28:
                        emit_bout2()
                    if i in (2, 4, 6):
                        emit_q(i // 2)
                sims = []
                for pr in range(2):
                    sim = psA.tile([P, 2, JW], f32, tag="sim")
                    for hh in range(2):
                        h = pr * 2 + hh
                        nc.tensor.matmul(
                            sim[:, hh, :],
                            lhsT=kT[32 * h:32 * h + 32, i * P:(i + 1) * P],
                            rhs=qT[32 * h:32 * h + 32, j * JW:(j + 1) * JW],
                            start=True, stop=True,
                            tile_position=(32 * h, 0),
                        )
                    sims.append(sim)
                # heads 0,1: exact exp on ACT; heads 2,3: fast-exp on DVE
                at0 = attnp.tile([P, 2, JW], b16, tag="at0", bufs=7)
                nc.scalar.activation(out=at0, in_=sims[0], func=AF.Exp,
                                     scale=SCALE)
                at1 = attnp.tile([P, 2, JW], u16, tag="at1", bufs=7)
                nc.vector.tensor_scalar(
                    out=at1, in0=sims[1], scalar1=FE_A, scalar2=FE_B,
                    op0=ALU.mult, op1=ALU.add,
                )
                pending.append((emit_avdn, i, at0, at1))
                if len(pending) > AVDELAY:
                    fn, ii, a0, a1 = pending.pop(0)
                    fn(ii, a0, a1)
                if epilogue is not None and i in EPI_AT:
                    epilogue(EPI_AT.index(i))
                    if i == EPI_AT[-1]:
                        epilogue = None
            epilogue = make_epilogue(j, oacc, dn)
        for fn, ii, a0, a1 in pending:
            fn(ii, a0, a1)
        epilogue.final()

    nc.finalize()
    _cache["nc"] = nc
    return nc


def _prep_in_maps(x, gn_weight, gn_bias, w_qkv, w_out, b_out):
    import ml_dtypes

    f = np.float32
    bf = ml_dtypes.bfloat16
    x = np.asarray(x, dtype=f).astype(bf)
    wqkvT = np.ascontiguousarray(np.asarray(w_qkv, dtype=f).T.astype(bf))
    woutT = np.ascontiguousarray(np.asarray(w_out, dtype=f).T.astype(bf))
    gnw = np.asarray(gn_weight, dtype=f).reshape(2, P)
    gnb = np.asarray(gn_bias, dtype=f).reshape(2, P)
    bo = np.asarray(b_out, dtype=f).reshape(2, P)
    ar = np.arange(P)
    # misc pack: cols 0-1 gnw(t0,t1), 2-3 gnb, 4-5 unused, 6-21 blk8
    misc = np.zeros((P, 22), f)
    misc[:, 0] = gnw[0]
    misc[:, 1] = gnw[1]
    misc[:, 2] = gnb[0]
    misc[:, 3] = gnb[1]
    misc[:, 4] = bo[0]
    misc[:, 5] = bo[1]
    misc[ar, 6 + ar // 8] = 0.125
    blk8T = np.zeros((16, P), f)
    blk8T[ar // 8, ar] = 1.0
    shared = dict(wqkvT=wqkvT, woutT=woutT, misc=misc, blk8T=blk8T)
    in_maps = []
    for core in range(NCORES):
        b, half = divmod(core, 2)
        xb = x[b].reshape(DIM, N)
        if half == 0:
            xp = np.ascontiguousarray(xb)
        else:
            xp = np.ascontiguousarray(
                np.concatenate([xb[:, NQ:], xb[:, :NQ]], axis=1)
            )
        in_maps.append(dict(x=xp, **shared))
    return in_maps


def _get_executor():
    """Build the sharded jitted executor once (compiles the NEFF once).

    Returns (exec_fn, meta): exec_fn takes a list of 8 per-core input dicts
    and returns the list of 8 per-core output dicts.  Mirrors
    concourse.bass2jax.run_bass_via_pjrt's multi-core path but caches the
    jax.jit so repeated calls don't recompile.
    """
    if "exec" in _cache:
        return _cache["exec"]
    import jax
    import concourse.mybir as mybir
    from jax.sharding import Mesh, PartitionSpec
    from jax.experimental.shard_map import shard_map
    from concourse import bass2jax

    bass2jax.install_neuronx_cc_hook()
    nc = _get_nc()

    partition_name = (
        nc.partition_id_tensor.name if nc.partition_id_tensor else None
    )
    in_names, out_names, out_avals, zero_outs = [], [], [], []
    for alloc in nc.m.functions[0].allocations:
        if not isinstance(alloc, mybir.MemoryLocationSet):
            continue
        name = alloc.memorylocations[0].name
        if alloc.kind == "ExternalInput":
            if name != partition_name:
                in_names.append(name)
        elif alloc.kind == "ExternalOutput":
            shape = tuple(alloc.tensor_shape)
            dtype = mybir.dt.np(alloc.dtype)
            out_names.append(name)
            out_avals.append(jax.core.ShapedArray(shape, dtype))
            zero_outs.append(np.zeros(shape, dtype))
    n_params = len(in_names)
    n_outs = len(out_names)
    all_names = in_names + out_names
    if partition_name is not None:
        all_names = all_names + [partition_name]

    def _body(*args):
        operands = list(args)
        if partition_name is not None:
            operands.append(bass2jax.partition_id_tensor())
        outs = bass2jax._bass_exec_p.bind(
            *operands,
            out_avals=tuple(out_avals),
            in_names=tuple(all_names),
            out_names=tuple(out_names),
            lowering_input_output_aliases=(),
            sim_require_finite=True,
            sim_require_nnan=True,
            nc=nc,
        )
        return tuple(outs)

    devices = jax.devices()[:NCORES]
    mesh = Mesh(np.asarray(devices), ("core",))
    sharded = jax.jit(
        shard_map(
            _body, mesh=mesh,
            in_specs=(PartitionSpec("core"),) * (n_params + n_outs),
            out_specs=(PartitionSpec("core"),) * n_outs,
            check_rep=False,
        ),
        keep_unused=True,
    )
    from jax.sharding import NamedSharding
    sharding = NamedSharding(mesh, PartitionSpec("core"))
    dev_zeros = [
        jax.device_put(
            np.zeros((NCORES * z.shape[0], *z.shape[1:]), z.dtype), sharding
        )
        for z in zero_outs
    ]

    def put_inputs(in_maps):
        return [
            jax.device_put(
                np.concatenate([np.asarray(m[name]) for m in in_maps], axis=0),
                sharding,
            )
            for name in in_names
        ]

    def run_device(device_inputs):
        return sharded(*device_inputs, *dev_zeros)

    def exec_fn(in_maps, device_inputs=None):
        if device_inputs is None:
            device_inputs = put_inputs(in_maps)
        out_arrs = [np.asarray(a) for a in run_device(device_inputs)]
        return [
            {
                name: out_arrs[i].reshape(NCORES, *out_avals[i].shape)[c]
                for i, name in enumerate(out_names)
            }
            for c in range(NCORES)
        ]

    meta = dict(in_names=in_names, out_names=out_names, mesh=mesh,
                sharded=sharded, zero_outs=zero_outs,
                put_inputs=put_inputs, run_device=run_device)
    _cache["exec"] = (exec_fn, meta)
    return _cache["exec"]


def _assemble(results):
    y = np.empty((4, DIM, N), np.float32)
    for core in range(NCORES):
        b, half = divmod(core, 2)
        y[b][:, half * NQ:(half + 1) * NQ] = results[core]["y"]
    return y.reshape(4, DIM, 64, 64)


def _run(inputs, **kw):
    exec_fn, _ = _get_executor()
    in_maps = _prep_in_maps(**inputs)
    results = exec_fn(in_maps)
    return _assemble(results), results


def kernel(x, gn_weight, gn_bias, w_qkv, w_out, b_out):
    out, _ = _run(dict(x=x, gn_weight=gn_weight, gn_bias=gn_bias,
                       w_qkv=w_qkv, w_out=w_out, b_out=b_out))
    return out
